# revision 15
# baseline (speedup 1.0000x reference)
"""Trainium2 Bass kernel for nn_Attention_68401649156342.

Reference computation (per batch element b of 8):
    q = MLP_q(x[b])                 # relu(x@Wq1+bq1)@Wq2+bq2 -> [2048,256]
    k = MLP_k(x[b])
    s = q @ k.T                     # [2048,2048]
    out[b] = softmax(s / rowmax(s), axis=-1)

Sharding: pure data-parallel over batch. Each of the 8 NeuronCores handles one
batch element end-to-end; no collectives.

Key algebraic restructure: since bq2 = bk2 = 0 (fixed by the problem spec),
    s = (h_q Wq2)(h_k Wk2)^T = h_q (Wq2 Wk2^T) h_k^T = (h_q M) h_k^T
with M = Wq2 @ Wk2.T precomputed in f32 on the host. This removes one of the
two second-layer matmuls and both q/k cast epilogues: h_k itself is the
scores RHS operand and g = h_q M is the scores LHS.

Numerics: fp16 everywhere (x, W1, M, h, g, p, o, output). Measured end-to-end
rel err ~3e-4 vs the f32 reference.

v2 changes vs the 101us baseline:
  - ssum = ssa+ssb moved from DVE to Pool (gpsimd.tensor_add): the DVE
    stream is [max0, max1, combine, recip] only (~2.6us/tile), ACT exp
    ~2.56us/tile, Pool add+normalize ~2.0us, PE 8 matmuls 1.73us (hot).
  - Prologue: host-contiguous W1 [F,2,D] / M [128,2,D] layouts (no DMA
    rearrange), input DMA issue order xT0,w1,bc,xT1,m, warmup memset on DVE.
  - PE p-state: warmup opens the clock during the input-DMA wait; the score
    pipeline then keeps PE gaps short so the 2.4GHz clock never drops
    (427ns -> 216ns matmuls).

Probed-and-rejected: tensor_tensor_reduce (any op combo) crashes the device
at runtime in this environment (custom DVE ucode); DVE/ACT instructions can
read only ONE PSUM operand; Pool (gpsimd) tensor ops cannot read PSUM at all
(InstTensorScalarPtr verifier reject) -- so L1/g epilogues stay on ACT/DVE.
"""

import os
from contextlib import ExitStack

import numpy as np

B, S, F, D = 8, 2048, 128, 256
NCORES = 8
H = 1024  # PSUM half-tile width

_CACHED = {}


def _build():
    import concourse.bass as bass
    import concourse.tile as tile
    from concourse import bacc, mybir

    f32 = mybir.dt.float32
    f16 = mybir.dt.float16
    AF = mybir.ActivationFunctionType
    OP = mybir.AluOpType

    nc = bacc.Bacc("TRN2", target_bir_lowering=False, debug=False,
                   num_devices=NCORES)

    xT_d = nc.dram_tensor("xT", [F, S], f16, kind="ExternalInput")
    # host-contiguous: [F, 2, D] (q/k stacked on middle axis)
    w1_d = nc.dram_tensor("W1", [F, 2, D], f16, kind="ExternalInput")
    # M = Wq2 @ Wk2.T, host-tiled: [128, ktile, D]
    m_d = nc.dram_tensor("M", [128, 2, D], f16, kind="ExternalInput")
    # per-partition constants: cols 0-1 = bq1[m], 2-3 = bk1[m], col 4 = -1.0
    bc_d = nc.dram_tensor("BC", [128, 8], f32, kind="ExternalInput")
    out_d = nc.dram_tensor("out", [S, S], f16, kind="ExternalOutput")

    NT = S // 128   # 16 score row-tiles

    with tile.TileContext(nc) as tc, ExitStack() as ctx:
        persist = ctx.enter_context(tc.tile_pool(name="persist", bufs=1))
        # 4 x [128,1024] f32 = all 8 PSUM banks (2 score tiles in flight).
        # NOTE: a 3-slot + keep-alive-bank variant was measured WORSE
        # (118us): constant dummy-matmul PSUM writes slow every PSUM read
        # 10-30% (exp 1095->1444ns) and 3 slots serialize exp->matmul.
        psum = ctx.enter_context(
            tc.tile_pool(name="psum", bufs=4, space="PSUM"))
        pp16 = ctx.enter_context(tc.tile_pool(name="pp16", bufs=1))
        pp32 = ctx.enter_context(tc.tile_pool(name="pp32", bufs=3))
        opool = ctx.enter_context(tc.tile_pool(name="opool", bufs=3))
        stats = ctx.enter_context(tc.tile_pool(name="stats", bufs=6))

        # ---- input DMAs: xT half 0 first (gates L1), then weights ----
        xTh = [persist.tile([F, H], f16, tag=f"xT{i}", name=f"xT{i}")
               for i in range(2)]
        w1 = persist.tile([F, 2, D], f16, tag="w1")
        bc = persist.tile([128, 8], f32, tag="bc")
        m_sb = persist.tile([128, 2, D], f16, tag="m_sb")

        nc.sync.dma_start(xTh[0][:], xT_d[:, 0:H])
        nc.sync.dma_start(w1[:], w1_d[:])
        nc.sync.dma_start(bc[:], bc_d[:])
        nc.sync.dma_start(xTh[1][:], xT_d[:, H:S])
        nc.sync.dma_start(m_sb[:], m_d[:])

        def b1sb(s, m):
            return bc[:, 2 * s + m:2 * s + m + 1]

        neg1 = bc[:, 4:5]

        # ---- PE warm-up: dummy matmuls run during the input-DMA wait so
        # the clock-gate opens before the first real matmul (the PE needs
        # ~3.4us of continuous execution to reach the 2.4GHz p-state) ----
        warm = persist.tile([128, 512], f16, tag="warm")
        nc.vector.memset(warm[:], 0.0)
        wps = psum.tile([128, H], f32, tag="ps", name="wps")
        for i in range(8):
            nc.tensor.matmul(wps[:, 0:512], warm[:, 0:128], warm[:],
                             start=True, stop=True)
        # dummy exp: pulls the ACT_TABLE_LOAD (1.3us) into the input-DMA
        # wait instead of delaying the first exp
        dumm = stats.tile([128, 1], f32, tag="dumm")
        nc.scalar.activation(dumm[:], warm[:, 0:1], AF.Exp,
                             bias=0.0, scale=1.0)

        # ---- L1 MLPs: hT[side][m][d, s] = relu(W1.T @ xT + b1), fp16 ----
        # q side first: the g matmul consumes h_q, while h_k is only needed
        # once scores start. Epilogues alternate ScalarE/DVE (Pool cannot
        # read PSUM).
        h = [[None, None], [None, None]]  # [side][m] -> [128, S]
        for s in (0, 1):
            for m in range(2):
                h[s][m] = persist.tile([F, S], f16, tag=f"h{s}{m}",
                                       name=f"h{s}{m}")
                for hf in range(2):
                    ps = psum.tile([128, H], f32, tag="ps")
                    for n in range(2):
                        nc.tensor.matmul(
                            ps[:, n * 512:(n + 1) * 512],
                            w1[:, s, m * 128:(m + 1) * 128],
                            xTh[hf][:, n * 512:(n + 1) * 512],
                            start=True, stop=True)
                    dst = h[s][m][:, hf * H:(hf + 1) * H]
                    if hf == 0:
                        nc.scalar.activation(dst, ps[:], AF.Relu,
                                             bias=b1sb(s, m), scale=1.0)
                    else:
                        nc.vector.tensor_scalar(dst, ps[:], b1sb(s, m), 0.0,
                                                OP.add, OP.max)
        hq, hk = h

        # ---- g = h_q @ M: gT quarter tiles [m2][hf] -> [f, 1024], fp16 ----
        # hf-outer order + separate quarter tiles: score tiles 0-7 only
        # depend on the hf=0 quarters, so scoring starts while the hf=1
        # half of g is still in the matmul.
        gq = [[None, None], [None, None]]  # [m2][hf]
        for hf in range(2):
            for m2 in range(2):
                gq[m2][hf] = persist.tile([128, H], f16, tag=f"g{m2}{hf}",
                                          name=f"g{m2}{hf}")
                ps2 = psum.tile([128, H], f32, tag="ps")
                for k in range(2):
                    for n in range(2):
                        nc.tensor.matmul(
                            ps2[:, n * 512:(n + 1) * 512],
                            m_sb[:, k, m2 * 128:(m2 + 1) * 128],
                            hq[k][:, hf * H + n * 512:hf * H + (n + 1) * 512],
                            start=(k == 0), stop=(k == 1))
                if m2 == 0:
                    nc.scalar.copy(gq[m2][hf][:], ps2[:])
                else:
                    nc.vector.tensor_copy(gq[m2][hf][:], ps2[:])

        # ---- scores + softmax, tile by tile ----
        def finish(pend):
            # deferred normalize: runs one tile behind so the engines that
            # feed the exp are never blocked by the previous tile's tail
            p, ssa, ssb, m, last = pend
            ssum = stats.tile([128, 1], f32, tag="ssum", name="ssum")
            o = opool.tile([128, S], f16, tag="o", name="o")
            if not last:
                # add on DVE: mixing op types on GpSimd thrashes its ucode
                # library (LIBRARY_RELOAD per switch, ~us each)
                nc.vector.tensor_add(ssum[:], ssa[:], ssb[:])
                # o = p/ssum and ssum <- 1/ssum in one Pool op (f32 p)
                nc.gpsimd.normalize_recip(o[:], p[:], ssum[:])
            else:
                # shortest tail chain: DVE add+recip+mul (fp16 p)
                nc.vector.tensor_add(ssum[:], ssa[:], ssb[:])
                rs = stats.tile([128, 1], f32, tag="rs", name="rs")
                nc.vector.reciprocal(rs[:], ssum[:])
                nc.vector.tensor_scalar_mul(o[:], p[:], rs[:])
            nc.sync.dma_start(out_d[m * 128:(m + 1) * 128, :], o[:])

        # Stats/exp run ONE TILE BEHIND the maxes: a big DVE reduce's
        # completion semaphore posts ~1.2us late, so a combine that waits on
        # maxb(T) right after it busy-stalls that long. With the 1-tile lag
        # the semaphore has the next tile's two 1.1us maxes as cover and
        # every small op issues instantly.
        def emit_stats_and_exp(prev):
            mxa, mxb, psh, m = prev
            mx = stats.tile([128, 1], f32, tag="mx", name="mx", bufs=1)
            nc.vector.tensor_max(mx[:], mxa[:], mxb[:])
            r = stats.tile([128, 1], f32, tag="r", name="r", bufs=3)
            nc.vector.reciprocal(r[:], mx[:])
            last = m == NT - 1
            if last:
                p = pp16.tile([128, S], f16, tag="p16", name="p16")
            else:
                p = pp32.tile([128, S], f32, tag="p32", name="p32")
            ssa = stats.tile([128, 1], f32, tag="ssa", name="ssa")
            ssb = stats.tile([128, 1], f32, tag="ssb", name="ssb")
            nc.scalar.activation(p[:, 0:H], psh[0][:], AF.Exp,
                                 bias=neg1, scale=r[:], accum_out=ssa[:])
            nc.scalar.activation(p[:, H:S], psh[1][:], AF.Exp,
                                 bias=neg1, scale=r[:], accum_out=ssb[:])
            return (p, ssa, ssb, m, last)

        prev = None   # (mxa, mxb, psh, m) awaiting combine/exp
        pend = None   # (p, ssa, ssb, m, last) awaiting normalize/DMA
        for m in range(NT):
            gcol = m // 8          # which g quarter holds this tile's rows
            goff = (m % 8) * 128
            psh = []
            for hf in range(2):
                ph = psum.tile([128, H], f32, tag="ps")
                for kk in range(2):
                    for n in range(2):
                        nc.tensor.matmul(
                            ph[:, n * 512:(n + 1) * 512],
                            gq[kk][gcol][:, goff:goff + 128],
                            hk[kk][:, hf * H + n * 512:hf * H + (n + 1) * 512],
                            start=(kk == 0), stop=(kk == 1))
                psh.append(ph)

            mxa = stats.tile([128, 1], f32, tag="mxa", name="mxa", bufs=2)
            mxb = stats.tile([128, 1], f32, tag="mxb", name="mxb", bufs=2)
            nc.vector.reduce_max(mxa[:], psh[0][:], axis=mybir.AxisListType.X)
            nc.vector.reduce_max(mxb[:], psh[1][:], axis=mybir.AxisListType.X)

            if prev is not None:
                new_pend = emit_stats_and_exp(prev)
                if pend is not None:
                    finish(pend)
                pend = new_pend
            prev = (mxa, mxb, psh, m)
        new_pend = emit_stats_and_exp(prev)
        if pend is not None:
            finish(pend)
        finish(new_pend)

    nc.compile()
    return nc


def _get_nc():
    if "nc" not in _CACHED:
        _CACHED["nc"] = _build()
    return _CACHED["nc"]


def _prep_inputs(x, Wq1, bq1, Wq2, bq2, Wk1, bk1, Wk2, bk2):
    # NOTE: bq2/bk2 are structurally zero for this problem (spec fill=zeros);
    # the W2-fold (M = Wq2 @ Wk2.T) relies on that.
    f16 = np.float16
    xT = np.ascontiguousarray(x.transpose(0, 2, 1)).astype(f16)  # [B,F,S]
    W1 = np.ascontiguousarray(np.stack([Wq1, Wk1], axis=1)).astype(f16)
    M = (np.asarray(Wq2, np.float32) @ np.asarray(Wk2, np.float32).T)
    M = np.ascontiguousarray(
        M.reshape(2, 128, D).transpose(1, 0, 2)).astype(f16)   # [128,2,D]
    BC = np.zeros((128, 8), np.float32)
    for s, b1v in enumerate([bq1, bk1]):
        for mm in range(2):
            BC[:, 2 * s + mm] = np.asarray(b1v)[mm * 128:(mm + 1) * 128]
    BC[:, 4] = -1.0
    BC[:, 5] = 1.0
    BC = np.ascontiguousarray(BC)
    return [
        {"xT": np.ascontiguousarray(xT[b]), "W1": W1, "M": M, "BC": BC}
        for b in range(B)
    ]


def _ensure_trace_hook():
    """Provide antenv.axon_hooks (NTFF profiling hook) if the image lacks it.

    Only matters when BASS_TRACE=1; degrades silently otherwise.
    """
    import sys
    import types
    try:
        import antenv.axon_hooks  # noqa: F401
        return
    except ImportError:
        pass
    try:
        import antenv
        from trn_agent_boot.trn_boot import _ntff_profile_via_ctypes

        mod = types.ModuleType("antenv.axon_hooks")
        state = {"hook": _ntff_profile_via_ctypes("/opt/axon/libaxon_pjrt.so")}
        mod.set_axon_ntff_profile_hook = lambda h: state.update(hook=h)
        mod.get_axon_ntff_profile_hook = lambda: state["hook"]
        sys.modules["antenv.axon_hooks"] = mod
        antenv.axon_hooks = mod
    except Exception:
        pass


def kernel(x, Wq1, bq1, Wq2, bq2, Wk1, bk1, Wk2, bk2):
    from concourse.bass_utils import run_bass_kernel_spmd

    try:
        _ensure_trace_hook()
    except Exception:
        pass

    nc = _get_nc()
    in_maps = _prep_inputs(x, Wq1, bq1, Wq2, bq2, Wk1, bk1, Wk2, bk2)
    res = run_bass_kernel_spmd(nc, in_maps, core_ids=list(range(NCORES)))
    _CACHED["last_results"] = res
    if res.exec_time_ns is not None:
        print(f"HW exec time: {res.exec_time_ns} ns")
    out = np.stack([res.results[b]["out"] for b in range(B)])
    # kernel computes/stores in fp16; deliver f32 to match the reference dtype
    return out.astype(np.float32)
